# revision 2
# baseline (speedup 1.0000x reference)
"""LIF (leaky integrate-and-fire) spiking recurrence on 8 Trainium2 cores.

Full input x: [T*bs, C, H, W] = [256, 128, 32, 32] f32 with T=8, bs=32.
Recurrence over T only, elementwise elsewhere:
    u_t = TAU * u_{t-1} * (1 - (u_{t-1} > VTH)) + x_t ;  o_t = (u_t > VTH)

Sharding: fully data-parallel over batch (bs=32 -> 4 per core), no collectives.

Per-core layout: SBUF tiles [C=128 partitions, 4*1024 free] per timestep.
Per step:
  DVE : u = (p * TAU) + x_t            (scalar_tensor_tensor, skipped at t=0: u=x_0)
  ACT : s = sign(u - VTH); o = relu(s) (exact 0/1: u-VTH is exact near VTH)
  DVE : p = (u <= VTH) * u             (scalar_tensor_tensor, skipped at t=T-1)
TAU=0.5 is a power of two and the masks are 0/1, so all arithmetic except the
final add is exact -> bitwise identical to the f32 reference.
"""

import numpy as np

import concourse.tile as tile
from concourse import bacc, mybir
from concourse.bass_utils import run_bass_kernel_spmd

T = 8
BS = 32
C = 128
HW = 32 * 32
NCORES = 8
BSH = BS // NCORES          # 4 batch elements per core
FREE = BSH * HW             # 4096 f32 per partition per timestep
VTH = 1.0
TAU = 0.5
F32 = mybir.dt.float32

_nc_cache = None


def _build():
    nc = bacc.Bacc("TRN2", target_bir_lowering=False, debug=False, num_devices=NCORES)
    x_d = nc.dram_tensor("x", [T, BSH, C, HW], F32, kind="ExternalInput").ap()
    o_d = nc.dram_tensor("o", [T, BSH, C, HW], F32, kind="ExternalOutput").ap()

    with tile.TileContext(nc) as tc:
        with (
            tc.tile_pool(name="xp", bufs=2) as xp,
            tc.tile_pool(name="up", bufs=2) as up,
            tc.tile_pool(name="pp", bufs=2) as pp,
            tc.tile_pool(name="sp", bufs=2) as sp,
            tc.tile_pool(name="op", bufs=2) as op,
        ):
            p = None
            for t in range(T):
                xt = xp.tile([C, FREE], F32)
                # HBM slab for (t, this core) is [b, c, hw]; land it as
                # [c partitions, b, hw] so C maps to the 128 partitions.
                nc.sync.dma_start(
                    out=xt[:].rearrange("c (b hw) -> c b hw", b=BSH),
                    in_=x_d[t].rearrange("b c hw -> c b hw"),
                )
                if t == 0:
                    u = xt  # u_0 = x_0 since u starts at 0
                else:
                    u = up.tile([C, FREE], F32)
                    nc.vector.scalar_tensor_tensor(
                        u[:], p[:], TAU, xt[:],
                        op0=mybir.AluOpType.mult, op1=mybir.AluOpType.add,
                    )
                # s = sign(VTH - u); o = relu(-s) = (u > VTH). Signs are
                # flipped via the scale immediate because only 0.0/1.0 have
                # pre-registered const APs for the bias operand.
                s = sp.tile([C, FREE], F32)
                nc.scalar.activation(
                    s[:], u[:], mybir.ActivationFunctionType.Sign,
                    bias=VTH, scale=-1.0,
                )
                o = op.tile([C, FREE], F32)
                nc.scalar.activation(
                    o[:], s[:], mybir.ActivationFunctionType.Relu, scale=-1.0
                )
                if t < T - 1:
                    p = pp.tile([C, FREE], F32)
                    nc.vector.scalar_tensor_tensor(
                        p[:], u[:], VTH, u[:],
                        op0=mybir.AluOpType.is_le, op1=mybir.AluOpType.mult,
                    )
                # Store from the ACT ring so it queues right behind the relu
                # without blocking the SP ring that feeds the next load.
                nc.scalar.dma_start(
                    out=o_d[t].rearrange("b c hw -> c b hw"),
                    in_=o[:].rearrange("c (b hw) -> c b hw", b=BSH),
                )

    nc.compile()
    return nc


def _get_nc():
    global _nc_cache
    if _nc_cache is None:
        _nc_cache = _build()
    return _nc_cache


def _run(x: np.ndarray, **spmd_kwargs):
    nc = _get_nc()
    xr = np.ascontiguousarray(np.asarray(x, dtype=np.float32)).reshape(T, BS, C, HW)
    in_maps = [
        {"x": np.ascontiguousarray(xr[:, k * BSH:(k + 1) * BSH])}
        for k in range(NCORES)
    ]
    res = run_bass_kernel_spmd(nc, in_maps, core_ids=list(range(NCORES)), **spmd_kwargs)
    out = np.empty((T, BS, C, HW), dtype=np.float32)
    for k in range(NCORES):
        out[:, k * BSH:(k + 1) * BSH] = res.results[k]["o"]
    return out.reshape(T * BS, C, 32, 32), res


def kernel(x: np.ndarray) -> np.ndarray:
    out, _ = _run(x)
    return out


# revision 5
# speedup vs baseline: 1.0979x; 1.0979x over previous
"""LIF (leaky integrate-and-fire) spiking recurrence on 8 Trainium2 cores.

Full input x: [T*bs, C, H, W] = [256, 128, 32, 32] f32 with T=8, bs=32.
Recurrence over T only, elementwise elsewhere:
    u_t = TAU * u_{t-1} * (1 - (u_{t-1} > VTH)) + x_t ;  o_t = (u_t > VTH)

Sharding: fully data-parallel over batch (bs=32 -> 4 per core), no collectives.

Since the op is elementwise outside of T, each core views its [4,128,32,32]
per-timestep slab as a flat [128 partitions, 4096] tile (16 KiB contiguous
HBM run per partition -> large DMA descriptors). Each timestep is split into
CH chunks so compute and output stores start as early as possible; the two
chunk chains interleave on DVE and hide cross-engine stalls.

Per step and chunk:
  DVE : u = (p * TAU) + x_t            (scalar_tensor_tensor; t=0: u=x_0)
  ACT : s = sign(VTH - u); o = relu(-s) = (u > VTH)   (exact: u-VTH exact near VTH)
  DVE : p = (u <= VTH) * u             (skipped at t=T-1)
TAU=0.5 is a power of two and the masks are 0/1, so everything except the
add is exact -> bitwise identical to the f32 reference.
"""

import numpy as np

import concourse.tile as tile
from concourse import bacc, mybir
from concourse.bass_utils import run_bass_kernel_spmd

T = 8
BS = 32
C = 128
HW = 32 * 32
NCORES = 8
BSH = BS // NCORES          # 4 batch elements per core
P = 128                     # SBUF partitions
FREE = BSH * C * HW // P    # 4096 f32 per partition per timestep
CH = 2                      # chunks per timestep
CHF = FREE // CH            # 2048
VTH = 1.0
TAU = 0.5
F32 = mybir.dt.float32

_nc_cache = None


def _build():
    nc = bacc.Bacc("TRN2", target_bir_lowering=False, debug=False, num_devices=NCORES)
    x_d = nc.dram_tensor("x", [T, P, FREE], F32, kind="ExternalInput").ap()
    o_d = nc.dram_tensor("o", [T, P, FREE], F32, kind="ExternalOutput").ap()

    with tile.TileContext(nc) as tc:
        with (
            tc.tile_pool(name="xp", bufs=8) as xp,
            tc.tile_pool(name="up", bufs=3) as up,
            tc.tile_pool(name="pp", bufs=2) as pp,
            tc.tile_pool(name="sp", bufs=2) as sp,
            tc.tile_pool(name="op", bufs=4) as op,
        ):
            p = [None] * CH
            for t in range(T):
                for c in range(CH):
                    sl = slice(c * CHF, (c + 1) * CHF)
                    xt = xp.tile([P, CHF], F32)
                    nc.sync.dma_start(out=xt[:], in_=x_d[t][:, sl])
                    if t == 0:
                        u = xt  # u_0 = x_0 since u starts at 0
                    else:
                        u = up.tile([P, CHF], F32)
                        nc.vector.scalar_tensor_tensor(
                            u[:], p[c][:], TAU, xt[:],
                            op0=mybir.AluOpType.mult, op1=mybir.AluOpType.add,
                        )
                    # s = sign(VTH - u); o = relu(-s) = (u > VTH). Signs are
                    # flipped via the scale immediate because only 0.0/1.0
                    # have pre-registered const APs for the bias operand.
                    s = sp.tile([P, CHF], F32)
                    nc.scalar.activation(
                        s[:], u[:], mybir.ActivationFunctionType.Sign,
                        bias=VTH, scale=-1.0,
                    )
                    o = op.tile([P, CHF], F32)
                    nc.scalar.activation(
                        o[:], s[:], mybir.ActivationFunctionType.Relu, scale=-1.0
                    )
                    if t < T - 1:
                        p[c] = pp.tile([P, CHF], F32, name="p", tag="p")
                        nc.vector.scalar_tensor_tensor(
                            p[c][:], u[:], VTH, u[:],
                            op0=mybir.AluOpType.is_le, op1=mybir.AluOpType.mult,
                        )
                    # Store from the ACT ring so it queues right behind the
                    # relu without blocking the SP ring feeding the loads.
                    nc.scalar.dma_start(out=o_d[t][:, sl], in_=o[:])

    nc.compile()
    return nc


def _get_nc():
    global _nc_cache
    if _nc_cache is None:
        _nc_cache = _build()
    return _nc_cache


def _run(x: np.ndarray, **spmd_kwargs):
    nc = _get_nc()
    xr = np.ascontiguousarray(np.asarray(x, dtype=np.float32)).reshape(T, BS, C, HW)
    in_maps = [
        {"x": np.ascontiguousarray(xr[:, k * BSH:(k + 1) * BSH]).reshape(T, P, FREE)}
        for k in range(NCORES)
    ]
    res = run_bass_kernel_spmd(nc, in_maps, core_ids=list(range(NCORES)), **spmd_kwargs)
    out = np.empty((T, BS, C, HW), dtype=np.float32)
    for k in range(NCORES):
        out[:, k * BSH:(k + 1) * BSH] = res.results[k]["o"].reshape(T, BSH, C, HW)
    return out.reshape(T * BS, C, 32, 32), res


def kernel(x: np.ndarray) -> np.ndarray:
    out, _ = _run(x)
    return out
